# revision 34
# baseline (speedup 1.0000x reference)
"""Distributed Trainium2 kernel for pre-LN multi-head self-attention.

Reference computation (n=2048, d=1024, 16 heads x 64):
    xn  = LayerNorm(x) * ln_scale + ln_bias
    qkv = xn @ w_qkv ; split -> q,k,v [16, 2048, 64]
    sim = (q @ k^T) * d**-0.5 ; attn = softmax(sim)
    out = concat_heads(attn @ v) @ w_out + b_out

Sharding: 2 heads per core (tensor parallel). Host ships x both row-major
(for LayerNorm stats) and pre-transposed (x^T, the matmul moving operand),
both in bf16, so the device never transposes x on the PE. Each core:
  - bn_stats on row-major x -> per-row mu, rstd; tiny PE transposes turn
    the per-tile stats columns into rows, a ones-matmul broadcasts rstd to
    all partitions, and DVE scales x^T by rstd in place
  - the -mu*rstd*colsum(W') LayerNorm cross-term is folded into the QKV
    matmuls as rank-1 accumulation matmuls (host ships negated column sums
    of the scale-folded weights); ln_bias terms fold into per-column biases
  - projects its 2 heads' q/k/v from the scaled x^T; v^T is transposed on
    the PE into row-major v with a ones column (softmax denominators)
  - attention in transposed layout over 9 row chunks (7x256 + 2x128),
    exp batched per key-chunk pair, accumulators double-buffered in PSUM
  - per-chunk AllGather of normalized head outputs, final projection one
    stage behind so the collective is fully overlapped
Host assembles the 8 [128, 2048] outT shards into the [2048, 1024] output.
"""

import sys

import ml_dtypes
import numpy as np

for _p in ("/opt/trn_rl_repo", "/root/.axon_site/_ro/trn_rl_repo"):
    if _p not in sys.path:
        sys.path.append(_p)

N = 2048          # sequence length
D = 1024          # model dim
HEADS = 16
DH = 64
NCORES = 8
HL = HEADS // NCORES          # heads per core (2)
HC = HL * DH                  # head cols per core (128)
LN_EPS = 1e-6
SIM_SCALE = float(D) ** -0.5  # reference scales by input dim

P = 128
RT = N // P        # 16 row tiles
DC = D // P        # 8 dim chunks
NBLK = 4           # 512-row blocks in phase 1
BW = N // NBLK     # 512

# attention row chunks: 7x256 + 2x128 (narrow tail hides the last AllGather)
CHUNKS = [(i * 256, 256) for i in range(7)] + [(1792, 128), (1920, 128)]
S = len(CHUNKS)

_BUILT = None
DEBUG = False


def _build():
    """Build the SPMD Bass graph (same graph on all 8 cores)."""
    from contextlib import ExitStack

    import concourse.tile as tile
    from concourse import bacc, mybir
    from concourse.masks import make_identity

    f32 = mybir.dt.float32
    bf16 = mybir.dt.bfloat16
    AF = mybir.ActivationFunctionType
    ALU = mybir.AluOpType

    nc = bacc.Bacc(None, num_devices=NCORES)

    xt_d = nc.declare_dram_parameter("xt", [D, N], bf16, isOutput=False)
    xr_d = nc.declare_dram_parameter("xr", [N, D], bf16, isOutput=False)
    wq_d = nc.declare_dram_parameter("wq", [D, HC], bf16, isOutput=False)
    wk_d = nc.declare_dram_parameter("wk", [D, HC], bf16, isOutput=False)
    wv_d = nc.declare_dram_parameter("wv", [D, HC], bf16, isOutput=False)
    ncsq_d = nc.declare_dram_parameter("ncsq", [1, HC], bf16, isOutput=False)
    ncsk_d = nc.declare_dram_parameter("ncsk", [1, HC], bf16, isOutput=False)
    ncsv_d = nc.declare_dram_parameter("ncsv", [1, HC], bf16, isOutput=False)
    qb_d = nc.declare_dram_parameter("qb", [HC], f32, isOutput=False)
    kb_d = nc.declare_dram_parameter("kb", [HC], f32, isOutput=False)
    vb_d = nc.declare_dram_parameter("vb", [HC], f32, isOutput=False)
    wo_d = nc.declare_dram_parameter("wo", [D, HC], bf16, isOutput=False)
    bo_d = nc.declare_dram_parameter("bo", [HC], f32, isOutput=False)
    out_d = nc.declare_dram_parameter("out", [HC, N], f32, isOutput=True)
    if DEBUG:
        dbg_d = {
            "dbg_rstd": nc.declare_dram_parameter(
                "dbg_rstd", [P, RT], bf16, isOutput=True),
            "dbg_mumr": nc.declare_dram_parameter(
                "dbg_mumr", [P, RT], bf16, isOutput=True),
            "dbg_xT": nc.declare_dram_parameter(
                "dbg_xT", [P, DC * N], bf16, isOutput=True),
            "dbg_qT": nc.declare_dram_parameter(
                "dbg_qT", [P, N], bf16, isOutput=True),
            "dbg_kT": nc.declare_dram_parameter(
                "dbg_kT", [P, N], bf16, isOutput=True),
            "dbg_vT": nc.declare_dram_parameter(
                "dbg_vT", [P, N], bf16, isOutput=True),
            "dbg_attn": nc.declare_dram_parameter(
                "dbg_attn", [DH, HL * N], bf16, isOutput=True),
            "dbg_po": nc.declare_dram_parameter(
                "dbg_po", [P, HL * 256], f32, isOutput=True),
            "dbg_den": nc.declare_dram_parameter(
                "dbg_den", [P, 256], f32, isOutput=True),
            "dbg_denb": nc.declare_dram_parameter(
                "dbg_denb", [P, 256], bf16, isOutput=True),
            "dbg_rb": nc.declare_dram_parameter(
                "dbg_rb", [DH, 256], f32, isOutput=True),
            "dbg_exp": nc.declare_dram_parameter(
                "dbg_exp", [P, (RT // 2) * HL * 512], bf16, isOutput=True),
            "dbg_rows": nc.declare_dram_parameter(
                "dbg_rows", [1, 2 * NBLK * P], bf16, isOutput=True),
            "dbg_seed": nc.declare_dram_parameter(
                "dbg_seed", [P, 512], f32, isOutput=True),
        }

    groups = [list(range(NCORES))]

    with ExitStack() as ctx:
        tc = ctx.enter_context(tile.TileContext(nc))

        dram = ctx.enter_context(tc.tile_pool(name="dram", bufs=1, space="DRAM"))
        # both heads stacked into one collective per chunk
        ag_in = [dram.tile([P, w], bf16, name=f"ag_in{i}")
                 for i, (_, w) in enumerate(CHUNKS)]
        ag_out = [dram.tile([NCORES * P, w], bf16, addr_space="Shared",
                            name=f"ag_out{i}") for i, (_, w) in enumerate(CHUNKS)]

        singles = ctx.enter_context(tc.tile_pool(name="singles", bufs=1))

        ident = singles.tile([P, P], bf16)
        make_identity(nc, ident)
        ones_sb = singles.tile([P, P], bf16)
        nc.vector.memset(ones_sb, 1.0)
        warm_rhs = singles.tile([P, 512], bf16)
        nc.vector.memset(warm_rhs, 0.0)
        eps_t = singles.tile([P, 1], f32)
        nc.vector.memset(eps_t, LN_EPS)

        # weights / biases / negated column sums (weight DMAs are emitted
        # inside phase 1, after block 0's stats load, to keep the sync DMA
        # queue's head startup-critical)
        wq_sb = singles.tile([P, DC, HC], bf16)
        wk_sb = singles.tile([P, DC, HC], bf16)
        wv_sb = singles.tile([P, DC, HC], bf16)
        wo_sb = singles.tile([P, DC, HC], bf16)
        ncsq_sb = singles.tile([1, HC], bf16)
        ncsk_sb = singles.tile([1, HC], bf16)
        ncsv_sb = singles.tile([1, HC], bf16)
        for c_sb, c_d in ((ncsk_sb, ncsk_d), (ncsq_sb, ncsq_d),
                          (ncsv_sb, ncsv_d)):
            nc.sync.dma_start(out=c_sb, in_=c_d[:, :])
        qb_t = singles.tile([P, 1], f32)
        kb_t = singles.tile([P, 1], f32)
        vb_t = singles.tile([P, 1], f32)
        bo_t = singles.tile([P, 1], f32)
        for b_t, b_d in ((qb_t, qb_d), (kb_t, kb_d), (vb_t, vb_d), (bo_t, bo_d)):
            nc.sync.dma_start(out=b_t, in_=b_d[:].rearrange("(p o) -> p o", o=1))

        # long-lived activations
        xT = singles.tile([P, DC, N], bf16)     # x^T, scaled in place by rstd
        qT = singles.tile([P, N], bf16)         # [2*64 qdims, rows]
        kT = singles.tile([P, N], bf16)
        vT = singles.tile([P, N], bf16)
        v_sb = singles.tile([P, RT, HL, DH + 1], bf16)  # [rowchunk, rt, h, v|1]
        attn_h = [singles.tile([DH, N], bf16, name=f"attn_h{h}")
                  for h in range(HL)]
        outT = singles.tile([P, N], f32)
        rstd16 = singles.tile([P, RT], bf16)    # per-row rstd, tiled [row%128, rt]
        mumr16 = singles.tile([P, RT], bf16)    # per-row mu*rstd

        nc.gpsimd.memset(v_sb[:, :, :, DH:], 1.0)  # ones column

        # x^T DMA on the Activation HWDGE queue (parallel with the sync
        # queue carrying x row-major + weights), one block at a time
        for blk in range(NBLK):
            nc.scalar.dma_start(
                out=xT[:, :, blk * BW:(blk + 1) * BW],
                in_=xt_d[:, blk * BW:(blk + 1) * BW].rearrange(
                    "(c p) m -> p c m", p=P),
            )

        # ---- phase 1: LN stats -> scale x^T -> q/k/v, per 512-row block ----
        with (
            tc.tile_pool(name="xrp", bufs=2) as xrp,
            tc.tile_pool(name="stat", bufs=8) as statp,
            tc.tile_pool(name="rowp", bufs=2) as rowp,
            tc.tile_pool(name="rbp", bufs=2) as rbp,
            tc.tile_pool(name="tp", bufs=2, space="PSUM") as tp,
            tc.tile_pool(name="pbp", bufs=1, space="PSUM") as pbp,
            tc.tile_pool(name="mmp", bufs=2, space="PSUM") as mmp,
        ):
            # short dependency-free matmul burst while the first DMAs land
            warm_ps = mmp.tile([P, BW], f32, tag="pm")
            for _ in range(12):
                nc.tensor.matmul(warm_ps, ident, warm_rhs,
                                 start=True, stop=True)

            def emit_stats(blk):
                """LN stats for one 512-row block (DVE + ACT only)."""
                xr_t = xrp.tile([P, NBLK, D], bf16, tag="xr")
                nc.sync.dma_start(
                    out=xr_t,
                    in_=xr_d[blk * BW:(blk + 1) * BW, :].rearrange(
                        "(a p) m -> p a m", p=P),
                )
                if blk == 0:
                    # weights ride the sync queue behind block 0's rows
                    for w_sb, w_d in ((wk_sb, wk_d), (wq_sb, wq_d),
                                      (wv_sb, wv_d), (wo_sb, wo_d)):
                        nc.sync.dma_start(
                            out=w_sb,
                            in_=w_d[:, :].rearrange("(c p) m -> p c m", p=P),
                        )
                for j in range(NBLK):
                    rt = blk * NBLK + j
                    stats = statp.tile([P, 2, 6], f32, tag="st")
                    for sg in range(2):
                        nc.vector.bn_stats(
                            out=stats[:, sg, :],
                            in_=xr_t[:, j, sg * 512:(sg + 1) * 512],
                        )
                    mv = statp.tile([P, 2], f32, tag="mv")
                    nc.vector.bn_aggr(out=mv, in_=stats)
                    rstd_f = statp.tile([P, 1], f32, tag="rstd")
                    nc.scalar.activation(
                        out=rstd_f, in_=mv[:, 1:2], func=AF.Sqrt,
                        bias=eps_t, scale=1.0,
                    )
                    nc.vector.reciprocal(out=rstd_f, in_=rstd_f)
                    with nc.allow_low_precision(reason="LN stats bf16 wire"):
                        nc.vector.tensor_copy(
                            out=rstd16[:, rt:rt + 1], in_=rstd_f
                        )
                        nc.vector.tensor_mul(
                            out=mumr16[:, rt:rt + 1],
                            in0=mv[:, 0:1], in1=rstd_f,
                        )

            emit_stats(0)
            for blk in range(NBLK):
                if blk + 1 < NBLK:
                    emit_stats(blk + 1)

                # transpose this block's per-tile stats columns into rows on
                # partition 0 (matmul operands need base partition 0/32/64)
                pt = tp.tile([1, 2, NBLK, P], bf16, tag="pt")
                with nc.allow_low_precision(reason="transpose copy"):
                    for j in range(NBLK):
                        rt = blk * NBLK + j
                        nc.tensor.transpose(
                            pt[:, 0, j, :], rstd16[:, rt:rt + 1], ident
                        )
                        nc.tensor.transpose(
                            pt[:, 1, j, :], mumr16[:, rt:rt + 1], ident
                        )
                rowsT = rowp.tile([1, 2, NBLK, P], bf16, tag="rows")
                with nc.allow_low_precision(reason="transpose copy"):
                    nc.scalar.copy(out=rowsT, in_=pt)

                if DEBUG and blk == 0:
                    nc.sync.dma_start(
                        out=dbg_d["dbg_rows"][:, :],
                        in_=rowsT[:].rearrange("o a j p -> o (a j p)"))
                    pdbg = mmp.tile([P, BW], f32, tag="pm")
                    for j in range(NBLK):
                        nc.tensor.matmul(
                            pdbg[:, j * P:(j + 1) * P],
                            ncsq_sb[0:1, :],
                            rowsT[0:1, 1, j, :],
                            start=True, stop=True,
                        )
                    sdbg = rowp.tile([P, BW], f32, tag="sdbg")
                    nc.vector.tensor_copy(out=sdbg, in_=pdbg)
                    nc.sync.dma_start(out=dbg_d["dbg_seed"][:, :], in_=sdbg)

                # broadcast rstd rows to all partitions via ones-matmuls
                pb = pbp.tile([P, BW], f32, tag="pb")
                for j in range(NBLK):
                    nc.tensor.matmul(
                        pb[:, j * P:(j + 1) * P],
                        ones_sb[0:1, :],
                        rowsT[0:1, 0, j, :],
                        start=True, stop=True,
                    )
                rb = rbp.tile([P, BW], bf16, tag="rb")
                with nc.allow_low_precision(reason="rstd bf16 wire"):
                    nc.scalar.copy(out=rb, in_=pb)

                # scale x^T by rstd in place (per dim chunk)
                cols = slice(blk * BW, (blk + 1) * BW)
                with nc.allow_low_precision(reason="xn bf16 wire"):
                    for kc in range(DC):
                        nc.vector.tensor_mul(
                            out=xT[:, kc, cols], in0=xT[:, kc, cols], in1=rb
                        )

                # q/k/v projections for this block; the -mu*rstd*colsum term
                # seeds the accumulation via rank-1 matmuls (contract dim 1)
                for w_sb, ncs_sb, b_t, dstT in (
                    (wk_sb, ncsk_sb, kb_t, kT),
                    (wq_sb, ncsq_sb, qb_t, qT),
                    (wv_sb, ncsv_sb, vb_t, vT),
                ):
                    pm = mmp.tile([P, BW], f32, tag="pm")
                    for kc in range(DC):
                        nc.tensor.matmul(
                            pm,
                            w_sb[:, kc, :],
                            xT[:, kc, cols],
                            start=(kc == 0), stop=False,
                        )
                    for j in range(NBLK):
                        nc.tensor.matmul(
                            pm[:, j * P:(j + 1) * P],
                            ncs_sb[0:1, :],
                            rowsT[0:1, 1, j, :],
                            start=False, stop=True,
                        )
                    nc.scalar.activation(
                        out=dstT[:, cols], in_=pm,
                        func=AF.Identity, bias=b_t, scale=1.0,
                    )

                # v^T -> v (row-major with ones column) for this block
                for j in range(NBLK):
                    rt = blk * NBLK + j
                    pv = tp.tile([P, P], bf16, tag="pv")
                    with nc.allow_low_precision(reason="transpose copy"):
                        nc.tensor.transpose(
                            pv, vT[:, rt * P:(rt + 1) * P], ident
                        )
                        nc.vector.tensor_copy(
                            out=v_sb[:, rt, :, 0:DH],
                            in_=pv[:].rearrange("p (h d) -> p h d", h=HL),
                        )

        # ---- phase 2: attention, software-pipelined across row chunks ----
        with (
            tc.tile_pool(name="expp", bufs=2) as expp,
            tc.tile_pool(name="rsum", bufs=4) as rsump,
            tc.tile_pool(name="sp", bufs=2, space="PSUM") as sp,
            tc.tile_pool(name="op", bufs=2, space="PSUM") as op,
            tc.tile_pool(name="rp", bufs=1, space="PSUM") as rp,
            tc.tile_pool(name="agp", bufs=2) as agp,
            tc.tile_pool(name="fp", bufs=1, space="PSUM") as fp,
        ):
            state = {}

            def sim_pair(idx, kcp):
                """Both heads' sim for key chunks 2*kcp, 2*kcp+1, one exp."""
                r0, w = CHUNKS[idx]
                st = state[idx]
                ps = sp.tile([P, HL, 2, 256], f32, tag="ps",
                             name=f"ps{idx}_{kcp}")
                for h in range(HL):
                    for t in range(2):
                        kc = 2 * kcp + t
                        nc.tensor.matmul(
                            ps[:, h, t, 0:w],
                            kT[h * DH:(h + 1) * DH, kc * P:(kc + 1) * P],
                            qT[h * DH:(h + 1) * DH, r0:r0 + w],
                            start=True, stop=True,
                        )
                if w == 256:
                    nc.scalar.activation(
                        out=st["exp_t"][:, kcp, :, :],
                        in_=ps[:].rearrange("p h t q -> p h (t q)"),
                        func=AF.Exp, scale=SIM_SCALE,
                    )
                else:
                    for t in range(2):
                        nc.scalar.activation(
                            out=st["exp_t"][:, kcp, :, t * 256:t * 256 + w],
                            in_=ps[:, :, t, 0:w],
                            func=AF.Exp, scale=SIM_SCALE,
                        )

            def av_pair(idx, kc):
                """attn@v for key chunk kc, both heads.

                start=True resets the whole PSUM bank's open accumulator, so
                the two heads (sharing one bank) must not each "start": open
                the bank once with a zeroing matmul, then only accumulate.
                """
                r0, w = CHUNKS[idx]
                st = state[idx]
                if st["po"] is None:
                    st["po"] = op.tile([P, HL, 256], f32, tag="po",
                                       name=f"po{idx}")
                    nc.tensor.matmul(
                        st["po"][:].rearrange("p h q -> p (h q)"),
                        ones_sb[0:1, :],
                        warm_rhs[0:1, :],
                        start=True, stop=False,
                    )
                for h in range(HL):
                    nc.tensor.matmul(
                        st["po"][0:DH + 1, h, 0:w],
                        v_sb[:, kc, h, :],
                        st["exp_t"][:, kc // 2, h,
                                    (kc % 2) * 256:(kc % 2) * 256 + w],
                        start=False, stop=(kc == RT - 1),
                    )

            def norm_tail(idx):
                """Normalize by softmax denominators, ship to the AG buffer."""
                r0, w = CHUNKS[idx]
                st = state[idx]
                po = st["po"]
                for h in range(HL):
                    den = rsump.tile([P, 256], f32, tag="den",
                                     name=f"den{idx}_{h}")
                    nc.vector.tensor_copy(
                        out=den[DH:DH + 1, 0:w], in_=po[DH:DH + 1, h, 0:w]
                    )
                    rec = rsump.tile([P, 256], f32, tag="rec",
                                     name=f"rec{idx}_{h}")
                    nc.vector.reciprocal(
                        out=rec[DH:DH + 1, 0:w], in_=den[DH:DH + 1, 0:w]
                    )
                    den_bf = rsump.tile([P, 256], bf16, tag="denb",
                                        name=f"denb{idx}_{h}")
                    with nc.allow_low_precision(reason="softmax recips"):
                        nc.vector.tensor_copy(
                            out=den_bf[DH:DH + 1, 0:w], in_=rec[DH:DH + 1, 0:w]
                        )
                    pr = rp.tile([DH, 256], f32, tag="pr", name=f"pr{idx}_{h}")
                    nc.tensor.matmul(
                        pr[:, 0:w], ones_sb[DH:DH + 1, 0:DH],
                        den_bf[DH:DH + 1, 0:w],
                        start=True, stop=True,
                    )
                    rb = rsump.tile([DH, 256], f32, tag="rb",
                                    name=f"rb{idx}_{h}")
                    nc.vector.tensor_copy(out=rb[:, 0:w], in_=pr[:, 0:w])
                    with nc.allow_low_precision(reason="attn bf16 wire"):
                        nc.vector.tensor_mul(
                            out=attn_h[h][:, r0:r0 + w],
                            in0=po[0:DH, h, 0:w], in1=rb[:, 0:w],
                        )
                    if DEBUG and idx == S - 1 and h == 0:
                        nc.sync.dma_start(out=dbg_d["dbg_den"][:, :], in_=den)
                        nc.sync.dma_start(
                            out=dbg_d["dbg_denb"][:, :], in_=den_bf)
                        po_sb = rsump.tile([P, HL * 256], f32, tag="dbgpo")
                        nc.vector.tensor_copy(
                            out=po_sb, in_=po[:].rearrange("p h q -> p (h q)"))
                        nc.sync.dma_start(out=dbg_d["dbg_po"][:, :], in_=po_sb)
                        rb_sb = rsump.tile([DH, 256], f32, tag="dbgrb")
                        nc.vector.tensor_copy(out=rb_sb, in_=rb)
                        nc.sync.dma_start(out=dbg_d["dbg_rb"][:, :], in_=rb_sb)
                        nc.sync.dma_start(
                            out=dbg_d["dbg_exp"][:, :],
                            in_=st["exp_t"][:].rearrange(
                                "p a h q -> p (a h q)"))
                    nc.gpsimd.dma_start(
                        out=ag_in[idx][h * DH:(h + 1) * DH, :],
                        in_=attn_h[h][:, r0:r0 + w],
                    )
                nc.gpsimd.collective_compute(
                    "AllGather",
                    mybir.AluOpType.bypass,
                    replica_groups=groups,
                    ins=[ag_in[idx][:].opt()],
                    outs=[ag_out[idx][:].opt()],
                )

            def proj(idx):
                """outT slice for this row chunk from the gathered heads."""
                r0, w = CHUNKS[idx]
                agt = agp.tile([P, DC, 256], bf16, tag="agt",
                               name=f"agt{idx}")
                for kc in range(DC):
                    nc.sync.dma_start(
                        out=agt[:, kc, 0:w],
                        in_=ag_out[idx][kc * P:(kc + 1) * P, :],
                    )
                pf = fp.tile([P, 256], f32, tag="pf", name=f"pf{idx}")
                for kc in range(DC):
                    nc.tensor.matmul(
                        pf[:, 0:w],
                        wo_sb[:, kc, :],
                        agt[:, kc, 0:w],
                        start=(kc == 0), stop=(kc == DC - 1),
                    )
                nc.vector.tensor_scalar(
                    out=outT[:, r0:r0 + w], in0=pf[:, 0:w],
                    scalar1=bo_t, scalar2=None,
                    op0=ALU.add,
                )
                nc.sync.dma_start(
                    out=out_d[:, r0:r0 + w], in_=outT[:, r0:r0 + w]
                )

            pending_proj = []
            for idx in range(S):
                state[idx] = {
                    "exp_t": expp.tile([P, RT // 2, HL, 512], bf16, tag="exp",
                                       name=f"exp{idx}"),
                    "po": None,
                }
                for kcp in range(RT // 2):
                    sim_pair(idx, kcp)
                    if idx > 0:
                        av_pair(idx - 1, 2 * kcp)
                        av_pair(idx - 1, 2 * kcp + 1)
                    if kcp == 3 and pending_proj:
                        proj(pending_proj.pop(0))
                if idx > 0:
                    norm_tail(idx - 1)
                    pending_proj.append(idx - 1)
            # drain: last chunk's av + norm + AG + remaining projections
            for kc in range(RT):
                av_pair(S - 1, kc)
                if kc in (5, 11) and pending_proj:
                    proj(pending_proj.pop(0))
            norm_tail(S - 1)
            for idx in pending_proj:
                proj(idx)
            proj(S - 1)

            if DEBUG:
                nc.sync.dma_start(out=dbg_d["dbg_rstd"][:, :], in_=rstd16)
                nc.sync.dma_start(out=dbg_d["dbg_mumr"][:, :], in_=mumr16)
                nc.sync.dma_start(
                    out=dbg_d["dbg_xT"][:, :],
                    in_=xT[:].rearrange("p c n -> p (c n)"))
                nc.sync.dma_start(out=dbg_d["dbg_qT"][:, :], in_=qT)
                nc.sync.dma_start(out=dbg_d["dbg_kT"][:, :], in_=kT)
                nc.sync.dma_start(out=dbg_d["dbg_vT"][:, :], in_=vT)
                for h in range(HL):
                    nc.sync.dma_start(
                        out=dbg_d["dbg_attn"][:, h * N:(h + 1) * N],
                        in_=attn_h[h][:, :])

    if not nc.is_finalized():
        nc.finalize()
    return nc


def _get_built():
    global _BUILT
    if _BUILT is None:
        _BUILT = _build()
    return _BUILT


def _shard_inputs(x, ln_scale, ln_bias, w_qkv, w_out, b_out):
    """Host-side sharding: slice per-head weight columns, fold LN params."""
    x = np.asarray(x, np.float32)
    ln_scale = np.asarray(ln_scale, np.float32)
    ln_bias = np.asarray(ln_bias, np.float32)
    w_qkv = np.asarray(w_qkv, np.float32)
    w_out = np.asarray(w_out, np.float32)
    b_out = np.asarray(b_out, np.float32)

    bf = ml_dtypes.bfloat16
    xt = np.ascontiguousarray(x.T.astype(bf))
    xr = np.ascontiguousarray(x.astype(bf))

    in_maps = []
    for ci in range(NCORES):
        c0 = ci * HC
        sl = {}
        for name, off in (("q", 0), ("k", HEADS * DH), ("v", 2 * HEADS * DH)):
            w = w_qkv[:, off + c0: off + c0 + HC]
            wp = ln_scale[:, None] * w
            sl["w" + name] = np.ascontiguousarray(wp.astype(bf))
            sl["ncs" + name] = np.ascontiguousarray(
                (-wp.sum(0)).astype(bf)[None, :]
            )
            sl[name + "b"] = np.ascontiguousarray(ln_bias @ w)
        sl["wo"] = np.ascontiguousarray(w_out[:, c0:c0 + HC].astype(bf))
        sl["bo"] = np.ascontiguousarray(b_out[c0:c0 + HC])
        sl["xt"] = xt
        sl["xr"] = xr
        in_maps.append(sl)
    return in_maps


def kernel(x, ln_scale, ln_bias, w_qkv, w_out, b_out):
    from concourse.bass_utils import run_bass_kernel_spmd

    nc = _get_built()
    in_maps = _shard_inputs(x, ln_scale, ln_bias, w_qkv, w_out, b_out)
    res = run_bass_kernel_spmd(nc, in_maps, core_ids=list(range(NCORES)))
    shards = [res.results[ci]["out"] for ci in range(NCORES)]  # [128, 2048] each
    outT = np.concatenate(shards, axis=0)  # [1024, 2048]
    return np.ascontiguousarray(outT.T)


# revision 46
# speedup vs baseline: 1.0647x; 1.0647x over previous
"""Distributed Trainium2 kernel for pre-LN multi-head self-attention.

Reference computation (n=2048, d=1024, 16 heads x 64):
    xn  = LayerNorm(x) * ln_scale + ln_bias
    qkv = xn @ w_qkv ; split -> q,k,v [16, 2048, 64]
    sim = (q @ k^T) * d**-0.5 ; attn = softmax(sim)
    out = concat_heads(attn @ v) @ w_out + b_out

Sharding: 2 heads per core (tensor parallel). Host ships x both row-major
(for LayerNorm stats) and pre-transposed (x^T, the matmul moving operand),
both in bf16, so the device never transposes x on the PE. Each core:
  - bn_stats on row-major x -> per-row mu, rstd; tiny PE transposes turn
    the per-tile stats columns into rows, a ones-matmul broadcasts rstd to
    all partitions, and DVE scales x^T by rstd in place
  - the -mu*rstd*colsum(W') LayerNorm cross-term is folded into the QKV
    matmuls as rank-1 accumulation matmuls (host ships negated column sums
    of the scale-folded weights); ln_bias terms fold into per-column biases
  - projects its 2 heads' q/k/v from the scaled x^T; v^T is transposed on
    the PE into row-major v with a ones column (softmax denominators)
  - attention in transposed layout over 9 row chunks (7x256 + 2x128),
    exp batched per key-chunk pair, accumulators double-buffered in PSUM
  - per-chunk AllGather of normalized head outputs, final projection one
    stage behind so the collective is fully overlapped
Host assembles the 8 [128, 2048] outT shards into the [2048, 1024] output.
"""

import sys

import ml_dtypes
import numpy as np

for _p in ("/opt/trn_rl_repo", "/root/.axon_site/_ro/trn_rl_repo"):
    if _p not in sys.path:
        sys.path.append(_p)

N = 2048          # sequence length
D = 1024          # model dim
HEADS = 16
DH = 64
NCORES = 8
HL = HEADS // NCORES          # heads per core (2)
HC = HL * DH                  # head cols per core (128)
LN_EPS = 1e-6
SIM_SCALE = float(D) ** -0.5  # reference scales by input dim

P = 128
RT = N // P        # 16 row tiles
DC = D // P        # 8 dim chunks
NBLK = 4           # 512-row blocks in phase 1
BW = N // NBLK     # 512

# attention row chunks: 7x256 + 2x128 (narrow tail hides the last AllGather)
CHUNKS = [(i * 256, 256) for i in range(7)] + [(1792, 128), (1920, 128)]
S = len(CHUNKS)

_BUILT = None
DEBUG = False


def _build():
    """Build the SPMD Bass graph (same graph on all 8 cores)."""
    from contextlib import ExitStack

    import concourse.tile as tile
    from concourse import bacc, mybir
    from concourse.masks import make_identity

    f32 = mybir.dt.float32
    bf16 = mybir.dt.bfloat16
    AF = mybir.ActivationFunctionType
    ALU = mybir.AluOpType

    nc = bacc.Bacc(None, num_devices=NCORES)

    xt_d = nc.declare_dram_parameter("xt", [D, N], bf16, isOutput=False)
    # row-major x slice for LayerNorm stats: each core owns N/NCORES rows
    xr_d = nc.declare_dram_parameter("xr", [N // NCORES, D], bf16,
                                     isOutput=False)
    wq_d = nc.declare_dram_parameter("wq", [D, HC], bf16, isOutput=False)
    wk_d = nc.declare_dram_parameter("wk", [D, HC], bf16, isOutput=False)
    wv_d = nc.declare_dram_parameter("wv", [D, HC], bf16, isOutput=False)
    ncsq_d = nc.declare_dram_parameter("ncsq", [1, HC], bf16, isOutput=False)
    ncsk_d = nc.declare_dram_parameter("ncsk", [1, HC], bf16, isOutput=False)
    ncsv_d = nc.declare_dram_parameter("ncsv", [1, HC], bf16, isOutput=False)
    qb_d = nc.declare_dram_parameter("qb", [HC], f32, isOutput=False)
    kb_d = nc.declare_dram_parameter("kb", [HC], f32, isOutput=False)
    vb_d = nc.declare_dram_parameter("vb", [HC], f32, isOutput=False)
    wo_d = nc.declare_dram_parameter("wo", [D, HC], bf16, isOutput=False)
    bo_d = nc.declare_dram_parameter("bo", [HC], f32, isOutput=False)
    out_d = nc.declare_dram_parameter("out", [HC, N], f32, isOutput=True)
    if DEBUG:
        dbg_d = {
            "dbg_xT": nc.declare_dram_parameter(
                "dbg_xT", [P, DC * N], bf16, isOutput=True),
            "dbg_qT": nc.declare_dram_parameter(
                "dbg_qT", [P, N], bf16, isOutput=True),
            "dbg_kT": nc.declare_dram_parameter(
                "dbg_kT", [P, N], bf16, isOutput=True),
            "dbg_vT": nc.declare_dram_parameter(
                "dbg_vT", [P, N], bf16, isOutput=True),
            "dbg_attn": nc.declare_dram_parameter(
                "dbg_attn", [DH, HL * N], bf16, isOutput=True),
            "dbg_po": nc.declare_dram_parameter(
                "dbg_po", [P, HL * 256], f32, isOutput=True),
            "dbg_den": nc.declare_dram_parameter(
                "dbg_den", [P, 256], f32, isOutput=True),
            "dbg_denb": nc.declare_dram_parameter(
                "dbg_denb", [P, 256], bf16, isOutput=True),
            "dbg_rb": nc.declare_dram_parameter(
                "dbg_rb", [DH, 256], f32, isOutput=True),
            "dbg_exp": nc.declare_dram_parameter(
                "dbg_exp", [P, (RT // 2) * HL * 512], bf16, isOutput=True),
            "dbg_rows": nc.declare_dram_parameter(
                "dbg_rows", [1, 2 * RT * P], bf16, isOutput=True),
        }

    groups = [list(range(NCORES))]

    with ExitStack() as ctx:
        tc = ctx.enter_context(tile.TileContext(nc))

        dram = ctx.enter_context(tc.tile_pool(name="dram", bufs=1, space="DRAM"))
        # both heads stacked into one collective per chunk
        ag_in = [dram.tile([P, w], bf16, name=f"ag_in{i}")
                 for i, (_, w) in enumerate(CHUNKS)]
        ag_out = [dram.tile([NCORES * P, w], bf16, addr_space="Shared",
                            name=f"ag_out{i}") for i, (_, w) in enumerate(CHUNKS)]
        # LN stats exchange: each core contributes [rstd|mumr] rows for its
        # N/NCORES rows
        st_in = dram.tile([2, N // NCORES], bf16, name="st_in")
        st_out = dram.tile([NCORES * 2, N // NCORES], bf16,
                           addr_space="Shared", name="st_out")

        singles = ctx.enter_context(tc.tile_pool(name="singles", bufs=1))

        ident = singles.tile([P, P], bf16)
        make_identity(nc, ident)
        ones_sb = singles.tile([P, P], bf16)
        nc.vector.memset(ones_sb, 1.0)
        warm_rhs = singles.tile([P, 512], bf16)
        nc.vector.memset(warm_rhs, 0.0)
        eps_t = singles.tile([P, 1], f32)
        nc.vector.memset(eps_t, LN_EPS)

        # weights / biases / negated column sums (weight DMAs are emitted
        # inside phase 1, after block 0's stats load, to keep the sync DMA
        # queue's head startup-critical)
        wq_sb = singles.tile([P, DC, HC], bf16)
        wk_sb = singles.tile([P, DC, HC], bf16)
        wv_sb = singles.tile([P, DC, HC], bf16)
        wo_sb = singles.tile([P, DC, HC], bf16)
        ncsq_sb = singles.tile([1, HC], bf16)
        ncsk_sb = singles.tile([1, HC], bf16)
        ncsv_sb = singles.tile([1, HC], bf16)
        for c_sb, c_d in ((ncsk_sb, ncsk_d), (ncsq_sb, ncsq_d),
                          (ncsv_sb, ncsv_d)):
            nc.sync.dma_start(out=c_sb, in_=c_d[:, :])
        qb_t = singles.tile([P, 1], f32)
        kb_t = singles.tile([P, 1], f32)
        vb_t = singles.tile([P, 1], f32)
        bo_t = singles.tile([P, 1], f32)
        for b_t, b_d in ((qb_t, qb_d), (kb_t, kb_d), (vb_t, vb_d), (bo_t, bo_d)):
            nc.sync.dma_start(out=b_t, in_=b_d[:].rearrange("(p o) -> p o", o=1))

        # long-lived activations
        xT = singles.tile([P, DC, N], bf16)     # x^T, scaled in place by rstd
        qT = singles.tile([P, N], bf16)         # [2*64 qdims, rows]
        kT = singles.tile([P, N], bf16)
        vT = singles.tile([P, N], bf16)
        v_sb = singles.tile([P, RT, HL, DH + 1], bf16)  # [rowchunk, rt, h, v|1]
        attn_h = [singles.tile([DH, N], bf16, name=f"attn_h{h}")
                  for h in range(HL)]
        outT = singles.tile([P, N], f32)

        nc.gpsimd.memset(v_sb[:, :, :, DH:], 1.0)  # ones column

        # Input DMAs, balanced across the two HWDGE queues (sync + ACT) in
        # consumption order: this core's stats rows first, then weights and
        # x^T blocks, wo (needed last) at the end.
        xr_t = singles.tile([P, 2, D], bf16)    # this core's 256 stats rows
        nc.sync.dma_start(
            out=xr_t, in_=xr_d[:, :].rearrange("(a p) m -> p a m", p=P)
        )
        nc.sync.dma_start(
            out=wk_sb, in_=wk_d[:, :].rearrange("(c p) m -> p c m", p=P)
        )
        for w_sb, w_d in ((wq_sb, wq_d), (wv_sb, wv_d)):
            nc.scalar.dma_start(
                out=w_sb, in_=w_d[:, :].rearrange("(c p) m -> p c m", p=P)
            )
        for blk, eng in ((0, nc.sync), (1, nc.scalar), (2, nc.sync),
                         (3, nc.scalar)):
            eng.dma_start(
                out=xT[:, :, blk * BW:(blk + 1) * BW],
                in_=xt_d[:, blk * BW:(blk + 1) * BW].rearrange(
                    "(c p) m -> p c m", p=P),
            )
        nc.scalar.dma_start(
            out=wo_sb, in_=wo_d[:, :].rearrange("(c p) m -> p c m", p=P)
        )

        # ---- phase 1: sharded LN stats -> scale x^T -> q/k/v ----
        with (
            tc.tile_pool(name="stat", bufs=4) as statp,
            tc.tile_pool(name="rbp", bufs=2) as rbp,
            tc.tile_pool(name="tp", bufs=2, space="PSUM") as tp,
            tc.tile_pool(name="pbp", bufs=1, space="PSUM") as pbp,
            tc.tile_pool(name="mmp", bufs=2, space="PSUM") as mmp,
        ):
            # short dependency-free matmul burst while the first DMAs land
            warm_ps = mmp.tile([P, BW], f32, tag="pm")
            for _ in range(20):
                nc.tensor.matmul(warm_ps[:, 0:256], ident, warm_rhs[:, 0:256],
                                 start=True, stop=True)

            # LN stats for this core's 2 row tiles -> [rstd|mumr] columns
            rm2 = statp.tile([P, 2, 2], bf16, tag="rm2")  # [p, q, j]
            for j in range(2):
                stats = statp.tile([P, 2, 6], f32, tag="st")
                for sg in range(2):
                    nc.vector.bn_stats(
                        out=stats[:, sg, :],
                        in_=xr_t[:, j, sg * 512:(sg + 1) * 512],
                    )
                mv = statp.tile([P, 2], f32, tag="mv")
                nc.vector.bn_aggr(out=mv, in_=stats)
                rstd_f = statp.tile([P, 1], f32, tag="rstd")
                nc.scalar.activation(
                    out=rstd_f, in_=mv[:, 1:2], func=AF.Sqrt,
                    bias=eps_t, scale=1.0,
                )
                nc.vector.reciprocal(out=rstd_f, in_=rstd_f)
                with nc.allow_low_precision(reason="LN stats bf16 wire"):
                    nc.vector.tensor_copy(out=rm2[:, 0, j:j + 1], in_=rstd_f)
                    nc.vector.tensor_mul(
                        out=rm2[:, 1, j:j + 1], in0=mv[:, 0:1], in1=rstd_f,
                    )

            # transpose the stats columns to rows on partition 0, exchange
            # across cores, land as rows_all[0, q, rt, :] for all 16 tiles
            pt = tp.tile([1, 2, 2, P], bf16, tag="pt")
            with nc.allow_low_precision(reason="transpose copy"):
                for q in range(2):
                    for j in range(2):
                        nc.tensor.transpose(
                            pt[:, q, j, :], rm2[:, q, j:j + 1], ident
                        )
            loc_rows = statp.tile([1, 2, 2, P], bf16, tag="locr")
            with nc.allow_low_precision(reason="transpose copy"):
                nc.scalar.copy(out=loc_rows, in_=pt)
            nc.gpsimd.dma_start(
                out=st_in[:, :],
                in_=loc_rows[:].rearrange("o q j p -> o (q j p)"),
            )
            nc.gpsimd.collective_compute(
                "AllGather",
                mybir.AluOpType.bypass,
                replica_groups=groups,
                ins=[st_in[:].opt()],
                outs=[st_out[:].opt()],
            )
            rows_all = singles.tile([1, 2, RT, P], bf16)
            for q in range(2):
                nc.sync.dma_start(
                    out=rows_all[:, q, :, :].rearrange(
                        "o (c j) p -> o c j p", j=2),
                    in_=st_out[:, :].rearrange(
                        "(c q) (j p) -> q c j p", q=2, p=P)[q],
                )

            for blk in range(NBLK):
                # broadcast rstd rows to all partitions via ones-matmuls
                pb = pbp.tile([P, BW], f32, tag="pb")
                for j in range(NBLK):
                    nc.tensor.matmul(
                        pb[:, j * P:(j + 1) * P],
                        ones_sb[0:1, :],
                        rows_all[0:1, 0, blk * NBLK + j, :],
                        start=True, stop=True,
                    )
                rb = rbp.tile([P, BW], bf16, tag="rb")
                with nc.allow_low_precision(reason="rstd bf16 wire"):
                    nc.scalar.copy(out=rb, in_=pb)

                # scale x^T by rstd in place (per dim chunk)
                cols = slice(blk * BW, (blk + 1) * BW)
                with nc.allow_low_precision(reason="xn bf16 wire"):
                    for kc in range(DC):
                        nc.vector.tensor_mul(
                            out=xT[:, kc, cols], in0=xT[:, kc, cols], in1=rb
                        )

                # q/k/v projections for this block; the -mu*rstd*colsum term
                # folds in via rank-1 accumulation matmuls (contract dim 1)
                for w_sb, ncs_sb, b_t, dstT in (
                    (wk_sb, ncsk_sb, kb_t, kT),
                    (wq_sb, ncsq_sb, qb_t, qT),
                    (wv_sb, ncsv_sb, vb_t, vT),
                ):
                    pm = mmp.tile([P, BW], f32, tag="pm")
                    for kc in range(DC):
                        nc.tensor.matmul(
                            pm,
                            w_sb[:, kc, :],
                            xT[:, kc, cols],
                            start=(kc == 0), stop=False,
                        )
                    for j in range(NBLK):
                        nc.tensor.matmul(
                            pm[:, j * P:(j + 1) * P],
                            ncs_sb[0:1, :],
                            rows_all[0:1, 1, blk * NBLK + j, :],
                            start=False, stop=True,
                        )
                    nc.scalar.activation(
                        out=dstT[:, cols], in_=pm,
                        func=AF.Identity, bias=b_t, scale=1.0,
                    )

                # v^T -> v (row-major with ones column) for this block
                for j in range(NBLK):
                    rt = blk * NBLK + j
                    pv = tp.tile([P, P], bf16, tag="pv")
                    with nc.allow_low_precision(reason="transpose copy"):
                        nc.tensor.transpose(
                            pv, vT[:, rt * P:(rt + 1) * P], ident
                        )
                        nc.vector.tensor_copy(
                            out=v_sb[:, rt, :, 0:DH],
                            in_=pv[:].rearrange("p (h d) -> p h d", h=HL),
                        )

        # ---- phase 2: attention, software-pipelined across row chunks ----
        with (
            tc.tile_pool(name="expp", bufs=2) as expp,
            tc.tile_pool(name="rsum", bufs=4) as rsump,
            tc.tile_pool(name="sp", bufs=2, space="PSUM") as sp,
            tc.tile_pool(name="op", bufs=2, space="PSUM") as op,
            tc.tile_pool(name="rp", bufs=1, space="PSUM") as rp,
            tc.tile_pool(name="agp", bufs=2) as agp,
            tc.tile_pool(name="fp", bufs=1, space="PSUM") as fp,
        ):
            state = {}

            def sim_pair(idx, kcp):
                """Both heads' sim for key chunks 2*kcp, 2*kcp+1, one exp."""
                r0, w = CHUNKS[idx]
                st = state[idx]
                ps = sp.tile([P, HL, 2, 256], f32, tag="ps",
                             name=f"ps{idx}_{kcp}")
                for h in range(HL):
                    for t in range(2):
                        kc = 2 * kcp + t
                        nc.tensor.matmul(
                            ps[:, h, t, 0:w],
                            kT[h * DH:(h + 1) * DH, kc * P:(kc + 1) * P],
                            qT[h * DH:(h + 1) * DH, r0:r0 + w],
                            start=True, stop=True,
                        )
                if w == 256:
                    nc.scalar.activation(
                        out=st["exp_t"][:, kcp, :, :],
                        in_=ps[:].rearrange("p h t q -> p h (t q)"),
                        func=AF.Exp, scale=SIM_SCALE,
                    )
                else:
                    for t in range(2):
                        nc.scalar.activation(
                            out=st["exp_t"][:, kcp, :, t * 256:t * 256 + w],
                            in_=ps[:, :, t, 0:w],
                            func=AF.Exp, scale=SIM_SCALE,
                        )

            def av_pair(idx, kc):
                """attn@v for key chunk kc, both heads.

                start=True resets the whole PSUM bank's open accumulator, so
                the two heads (sharing one bank) must not each "start": open
                the bank once with a zeroing matmul, then only accumulate.
                """
                r0, w = CHUNKS[idx]
                st = state[idx]
                if st["po"] is None:
                    st["po"] = op.tile([P, HL, 256], f32, tag="po",
                                       name=f"po{idx}")
                    nc.tensor.matmul(
                        st["po"][:].rearrange("p h q -> p (h q)"),
                        ones_sb[0:1, :],
                        warm_rhs[0:1, :],
                        start=True, stop=False,
                    )
                for h in range(HL):
                    nc.tensor.matmul(
                        st["po"][0:DH + 1, h, 0:w],
                        v_sb[:, kc, h, :],
                        st["exp_t"][:, kc // 2, h,
                                    (kc % 2) * 256:(kc % 2) * 256 + w],
                        start=False, stop=(kc == RT - 1),
                    )

            def norm_tail(idx):
                """Normalize by softmax denominators, ship to the AG buffer."""
                r0, w = CHUNKS[idx]
                st = state[idx]
                po = st["po"]
                for h in range(HL):
                    den = rsump.tile([P, 256], f32, tag="den",
                                     name=f"den{idx}_{h}")
                    nc.vector.tensor_copy(
                        out=den[DH:DH + 1, 0:w], in_=po[DH:DH + 1, h, 0:w]
                    )
                    rec = rsump.tile([P, 256], f32, tag="rec",
                                     name=f"rec{idx}_{h}")
                    nc.vector.reciprocal(
                        out=rec[DH:DH + 1, 0:w], in_=den[DH:DH + 1, 0:w]
                    )
                    den_bf = rsump.tile([P, 256], bf16, tag="denb",
                                        name=f"denb{idx}_{h}")
                    with nc.allow_low_precision(reason="softmax recips"):
                        nc.vector.tensor_copy(
                            out=den_bf[DH:DH + 1, 0:w], in_=rec[DH:DH + 1, 0:w]
                        )
                    pr = rp.tile([DH, 256], f32, tag="pr", name=f"pr{idx}_{h}")
                    nc.tensor.matmul(
                        pr[:, 0:w], ones_sb[DH:DH + 1, 0:DH],
                        den_bf[DH:DH + 1, 0:w],
                        start=True, stop=True,
                    )
                    rb = rsump.tile([DH, 256], f32, tag="rb",
                                    name=f"rb{idx}_{h}")
                    nc.vector.tensor_copy(out=rb[:, 0:w], in_=pr[:, 0:w])
                    with nc.allow_low_precision(reason="attn bf16 wire"):
                        nc.vector.tensor_mul(
                            out=attn_h[h][:, r0:r0 + w],
                            in0=po[0:DH, h, 0:w], in1=rb[:, 0:w],
                        )
                    if DEBUG and idx == S - 1 and h == 0:
                        nc.sync.dma_start(out=dbg_d["dbg_den"][:, :], in_=den)
                        nc.sync.dma_start(
                            out=dbg_d["dbg_denb"][:, :], in_=den_bf)
                        po_sb = rsump.tile([P, HL * 256], f32, tag="dbgpo")
                        nc.vector.tensor_copy(
                            out=po_sb, in_=po[:].rearrange("p h q -> p (h q)"))
                        nc.sync.dma_start(out=dbg_d["dbg_po"][:, :], in_=po_sb)
                        rb_sb = rsump.tile([DH, 256], f32, tag="dbgrb")
                        nc.vector.tensor_copy(out=rb_sb, in_=rb)
                        nc.sync.dma_start(out=dbg_d["dbg_rb"][:, :], in_=rb_sb)
                        nc.sync.dma_start(
                            out=dbg_d["dbg_exp"][:, :],
                            in_=st["exp_t"][:].rearrange(
                                "p a h q -> p (a h q)"))
                    nc.gpsimd.dma_start(
                        out=ag_in[idx][h * DH:(h + 1) * DH, :],
                        in_=attn_h[h][:, r0:r0 + w],
                    )
                nc.gpsimd.collective_compute(
                    "AllGather",
                    mybir.AluOpType.bypass,
                    replica_groups=groups,
                    ins=[ag_in[idx][:].opt()],
                    outs=[ag_out[idx][:].opt()],
                )

            def proj(idx):
                """outT slice for this row chunk from the gathered heads."""
                r0, w = CHUNKS[idx]
                agt = agp.tile([P, DC, 256], bf16, tag="agt",
                               name=f"agt{idx}")
                for kc in range(DC):
                    nc.sync.dma_start(
                        out=agt[:, kc, 0:w],
                        in_=ag_out[idx][kc * P:(kc + 1) * P, :],
                    )
                pf = fp.tile([P, 256], f32, tag="pf", name=f"pf{idx}")
                for kc in range(DC):
                    nc.tensor.matmul(
                        pf[:, 0:w],
                        wo_sb[:, kc, :],
                        agt[:, kc, 0:w],
                        start=(kc == 0), stop=(kc == DC - 1),
                    )
                nc.vector.tensor_scalar(
                    out=outT[:, r0:r0 + w], in0=pf[:, 0:w],
                    scalar1=bo_t, scalar2=None,
                    op0=ALU.add,
                )
                nc.sync.dma_start(
                    out=out_d[:, r0:r0 + w], in_=outT[:, r0:r0 + w]
                )

            pending_proj = []
            for idx in range(S):
                state[idx] = {
                    "exp_t": expp.tile([P, RT // 2, HL, 512], bf16, tag="exp",
                                       name=f"exp{idx}"),
                    "po": None,
                }
                for kcp in range(RT // 2):
                    sim_pair(idx, kcp)
                    if idx > 0:
                        av_pair(idx - 1, 2 * kcp)
                        av_pair(idx - 1, 2 * kcp + 1)
                    if kcp == 3 and pending_proj:
                        proj(pending_proj.pop(0))
                if idx > 0:
                    norm_tail(idx - 1)
                    pending_proj.append(idx - 1)
            # drain: last chunk's av + norm + AG + remaining projections
            for kc in range(RT):
                av_pair(S - 1, kc)
                if kc in (5, 11) and pending_proj:
                    proj(pending_proj.pop(0))
            norm_tail(S - 1)
            for idx in pending_proj:
                proj(idx)
            proj(S - 1)

            if DEBUG:
                nc.sync.dma_start(
                    out=dbg_d["dbg_rows"][:, :],
                    in_=rows_all[:].rearrange("o q t p -> o (q t p)"))
                nc.sync.dma_start(
                    out=dbg_d["dbg_xT"][:, :],
                    in_=xT[:].rearrange("p c n -> p (c n)"))
                nc.sync.dma_start(out=dbg_d["dbg_qT"][:, :], in_=qT)
                nc.sync.dma_start(out=dbg_d["dbg_kT"][:, :], in_=kT)
                nc.sync.dma_start(out=dbg_d["dbg_vT"][:, :], in_=vT)
                for h in range(HL):
                    nc.sync.dma_start(
                        out=dbg_d["dbg_attn"][:, h * N:(h + 1) * N],
                        in_=attn_h[h][:, :])

    if not nc.is_finalized():
        nc.finalize()
    return nc


def _get_built():
    global _BUILT
    if _BUILT is None:
        _BUILT = _build()
    return _BUILT


def _shard_inputs(x, ln_scale, ln_bias, w_qkv, w_out, b_out):
    """Host-side sharding: slice per-head weight columns, fold LN params."""
    x = np.asarray(x, np.float32)
    ln_scale = np.asarray(ln_scale, np.float32)
    ln_bias = np.asarray(ln_bias, np.float32)
    w_qkv = np.asarray(w_qkv, np.float32)
    w_out = np.asarray(w_out, np.float32)
    b_out = np.asarray(b_out, np.float32)

    bf = ml_dtypes.bfloat16
    xt = np.ascontiguousarray(x.T.astype(bf))
    xr = np.ascontiguousarray(x.astype(bf))
    rows_per_core = N // NCORES

    in_maps = []
    for ci in range(NCORES):
        c0 = ci * HC
        sl = {}
        for name, off in (("q", 0), ("k", HEADS * DH), ("v", 2 * HEADS * DH)):
            w = w_qkv[:, off + c0: off + c0 + HC]
            wp = ln_scale[:, None] * w
            sl["w" + name] = np.ascontiguousarray(wp.astype(bf))
            sl["ncs" + name] = np.ascontiguousarray(
                (-wp.sum(0)).astype(bf)[None, :]
            )
            sl[name + "b"] = np.ascontiguousarray(ln_bias @ w)
        sl["wo"] = np.ascontiguousarray(w_out[:, c0:c0 + HC].astype(bf))
        sl["bo"] = np.ascontiguousarray(b_out[c0:c0 + HC])
        sl["xt"] = xt
        sl["xr"] = np.ascontiguousarray(
            xr[ci * rows_per_core:(ci + 1) * rows_per_core])
        in_maps.append(sl)
    return in_maps


def kernel(x, ln_scale, ln_bias, w_qkv, w_out, b_out):
    from concourse.bass_utils import run_bass_kernel_spmd

    nc = _get_built()
    in_maps = _shard_inputs(x, ln_scale, ln_bias, w_qkv, w_out, b_out)
    res = run_bass_kernel_spmd(nc, in_maps, core_ids=list(range(NCORES)))
    shards = [res.results[ci]["out"] for ci in range(NCORES)]  # [128, 2048] each
    outT = np.concatenate(shards, axis=0)  # [1024, 2048]
    return np.ascontiguousarray(outT.T)


# revision 57
# speedup vs baseline: 1.1615x; 1.0909x over previous
"""Distributed Trainium2 kernel for pre-LN multi-head self-attention.

Reference computation (n=2048, d=1024, 16 heads x 64):
    xn  = LayerNorm(x) * ln_scale + ln_bias
    qkv = xn @ w_qkv ; split -> q,k,v [16, 2048, 64]
    sim = (q @ k^T) * d**-0.5 ; attn = softmax(sim)
    out = concat_heads(attn @ v) @ w_out + b_out

Sharding: 2 heads per core (tensor parallel). Host ships x both row-major
(for LayerNorm stats) and pre-transposed (x^T, the matmul moving operand),
both in bf16, so the device never transposes x on the PE. Each core:
  - bn_stats on row-major x -> per-row mu, rstd; tiny PE transposes turn
    the per-tile stats columns into rows, a ones-matmul broadcasts rstd to
    all partitions, and DVE scales x^T by rstd in place
  - the -mu*rstd*colsum(W') LayerNorm cross-term is folded into the QKV
    matmuls as rank-1 accumulation matmuls (host ships negated column sums
    of the scale-folded weights); ln_bias terms fold into per-column biases
  - projects its 2 heads' q/k/v from the scaled x^T; v^T is transposed on
    the PE into row-major v with a ones column (softmax denominators)
  - attention in transposed layout over 9 row chunks (7x256 + 2x128),
    exp batched per key-chunk pair, accumulators double-buffered in PSUM
  - per-chunk AllGather of normalized head outputs, final projection one
    stage behind so the collective is fully overlapped
Host assembles the 8 [128, 2048] outT shards into the [2048, 1024] output.
"""

import sys

import ml_dtypes
import numpy as np

for _p in ("/opt/trn_rl_repo", "/root/.axon_site/_ro/trn_rl_repo"):
    if _p not in sys.path:
        sys.path.append(_p)

N = 2048          # sequence length
D = 1024          # model dim
HEADS = 16
DH = 64
NCORES = 8
HL = HEADS // NCORES          # heads per core (2)
HC = HL * DH                  # head cols per core (128)
LN_EPS = 1e-6
SIM_SCALE = float(D) ** -0.5  # reference scales by input dim

P = 128
RT = N // P        # 16 row tiles
DC = D // P        # 8 dim chunks
NBLK = 4           # 512-row blocks in phase 1
BW = N // NBLK     # 512

# attention row chunks: 7x256 + 2x128 (narrow tail hides the last AllGather)
CHUNKS = [(i * 256, 256) for i in range(7)] + [(1792, 128), (1920, 128)]
S = len(CHUNKS)

_BUILT = None
DEBUG = False


def _build():
    """Build the SPMD Bass graph (same graph on all 8 cores)."""
    from contextlib import ExitStack

    import concourse.tile as tile
    from concourse import bacc, mybir
    from concourse.masks import make_identity

    f32 = mybir.dt.float32
    bf16 = mybir.dt.bfloat16
    AF = mybir.ActivationFunctionType
    ALU = mybir.AluOpType

    nc = bacc.Bacc(None, num_devices=NCORES)

    xt_d = nc.declare_dram_parameter("xt", [D, N], bf16, isOutput=False)
    wq_d = nc.declare_dram_parameter("wq", [D, HC], bf16, isOutput=False)
    wk_d = nc.declare_dram_parameter("wk", [D, HC], bf16, isOutput=False)
    wv_d = nc.declare_dram_parameter("wv", [D, HC], bf16, isOutput=False)
    ncsq_d = nc.declare_dram_parameter("ncsq", [1, HC], bf16, isOutput=False)
    ncsk_d = nc.declare_dram_parameter("ncsk", [1, HC], bf16, isOutput=False)
    ncsv_d = nc.declare_dram_parameter("ncsv", [1, HC], bf16, isOutput=False)
    qb_d = nc.declare_dram_parameter("qb", [HC], f32, isOutput=False)
    kb_d = nc.declare_dram_parameter("kb", [HC], f32, isOutput=False)
    vb_d = nc.declare_dram_parameter("vb", [HC], f32, isOutput=False)
    wo_d = nc.declare_dram_parameter("wo", [D, HC], bf16, isOutput=False)
    bo_d = nc.declare_dram_parameter("bo", [HC], f32, isOutput=False)
    out_d = nc.declare_dram_parameter("out", [HC, N], f32, isOutput=True)
    if DEBUG:
        dbg_d = {
            "dbg_xT": nc.declare_dram_parameter(
                "dbg_xT", [P, DC * N], bf16, isOutput=True),
            "dbg_qT": nc.declare_dram_parameter(
                "dbg_qT", [P, N], bf16, isOutput=True),
            "dbg_kT": nc.declare_dram_parameter(
                "dbg_kT", [P, N], bf16, isOutput=True),
            "dbg_vT": nc.declare_dram_parameter(
                "dbg_vT", [P, N], bf16, isOutput=True),
            "dbg_attn": nc.declare_dram_parameter(
                "dbg_attn", [DH, HL * N], bf16, isOutput=True),
            "dbg_po": nc.declare_dram_parameter(
                "dbg_po", [P, HL * 256], f32, isOutput=True),
            "dbg_den": nc.declare_dram_parameter(
                "dbg_den", [P, 256], f32, isOutput=True),
            "dbg_denb": nc.declare_dram_parameter(
                "dbg_denb", [P, 256], bf16, isOutput=True),
            "dbg_rb": nc.declare_dram_parameter(
                "dbg_rb", [DH, 256], f32, isOutput=True),
            "dbg_exp": nc.declare_dram_parameter(
                "dbg_exp", [P, (RT // 2) * HL * 512], bf16, isOutput=True),
            "dbg_rows": nc.declare_dram_parameter(
                "dbg_rows", [1, 2 * RT * P], bf16, isOutput=True),
        }

    groups = [list(range(NCORES))]

    with ExitStack() as ctx:
        tc = ctx.enter_context(tile.TileContext(nc))

        dram = ctx.enter_context(tc.tile_pool(name="dram", bufs=1, space="DRAM"))
        # both heads stacked into one collective per chunk
        ag_in = [dram.tile([P, w], bf16, name=f"ag_in{i}")
                 for i, (_, w) in enumerate(CHUNKS)]
        ag_out = [dram.tile([NCORES * P, w], bf16, addr_space="Shared",
                            name=f"ag_out{i}") for i, (_, w) in enumerate(CHUNKS)]

        singles = ctx.enter_context(tc.tile_pool(name="singles", bufs=1))

        ident = singles.tile([P, P], bf16)
        make_identity(nc, ident)
        ones_sb = singles.tile([P, P], bf16)
        nc.vector.memset(ones_sb, 1.0)
        warm_rhs = singles.tile([P, 512], bf16)
        nc.vector.memset(warm_rhs, 0.0)
        eps_t = singles.tile([P, 1], f32)
        nc.vector.memset(eps_t, LN_EPS)

        # weights / biases / negated column sums (weight DMAs are emitted
        # inside phase 1, after block 0's stats load, to keep the sync DMA
        # queue's head startup-critical)
        wq_sb = singles.tile([P, DC, HC], bf16)
        wk_sb = singles.tile([P, DC, HC], bf16)
        wv_sb = singles.tile([P, DC, HC], bf16)
        wo_sb = singles.tile([P, DC, HC], bf16)
        ncsq_sb = singles.tile([1, HC], bf16)
        ncsk_sb = singles.tile([1, HC], bf16)
        ncsv_sb = singles.tile([1, HC], bf16)
        for c_sb, c_d in ((ncsk_sb, ncsk_d), (ncsq_sb, ncsq_d),
                          (ncsv_sb, ncsv_d)):
            nc.sync.dma_start(out=c_sb, in_=c_d[:, :])
        qb_t = singles.tile([P, 1], f32)
        kb_t = singles.tile([P, 1], f32)
        vb_t = singles.tile([P, 1], f32)
        bo_t = singles.tile([P, 1], f32)
        for b_t, b_d in ((qb_t, qb_d), (kb_t, kb_d), (vb_t, vb_d), (bo_t, bo_d)):
            nc.sync.dma_start(out=b_t, in_=b_d[:].rearrange("(p o) -> p o", o=1))

        # long-lived activations
        xT = singles.tile([P, DC, N], bf16)     # x^T, scaled in place by rstd
        qT = singles.tile([P, N], bf16)         # [2*64 qdims, rows]
        kT = singles.tile([P, N], bf16)
        vT = singles.tile([P, N], bf16)
        v_sb = singles.tile([P, RT, HL, DH + 1], bf16)  # [rowchunk, rt, h, v|1]
        attn_h = [singles.tile([DH, N], bf16, name=f"attn_h{h}")
                  for h in range(HL)]
        outT = singles.tile([P, N], f32)

        nc.gpsimd.memset(v_sb[:, :, :, DH:], 1.0)  # ones column

        # Input DMAs, balanced across the two HWDGE queues (sync + ACT) in
        # consumption order: x^T blocks lead, wo (needed last) at the end.
        for blk, eng in ((0, nc.sync), (1, nc.scalar)):
            eng.dma_start(
                out=xT[:, :, blk * BW:(blk + 1) * BW],
                in_=xt_d[:, blk * BW:(blk + 1) * BW].rearrange(
                    "(c p) m -> p c m", p=P),
            )
        nc.sync.dma_start(
            out=wk_sb, in_=wk_d[:, :].rearrange("(c p) m -> p c m", p=P)
        )
        for w_sb, w_d in ((wq_sb, wq_d), (wv_sb, wv_d)):
            nc.scalar.dma_start(
                out=w_sb, in_=w_d[:, :].rearrange("(c p) m -> p c m", p=P)
            )
        for blk, eng in ((2, nc.sync), (3, nc.scalar)):
            eng.dma_start(
                out=xT[:, :, blk * BW:(blk + 1) * BW],
                in_=xt_d[:, blk * BW:(blk + 1) * BW].rearrange(
                    "(c p) m -> p c m", p=P),
            )
        nc.scalar.dma_start(
            out=wo_sb, in_=wo_d[:, :].rearrange("(c p) m -> p c m", p=P)
        )

        # ---- phase 1: matmul LN stats -> scale x^T -> q/k/v ----
        # Per-row sums of x and x^2 come from ones-stationary matmuls over
        # x^T; var -> rstd via ln/exp on ACT (exp(-0.5*ln(var+eps))), which
        # shares the activation table with the attention exps (no reloads).
        with (
            tc.tile_pool(name="rowp", bufs=2) as rowp,
            tc.tile_pool(name="sqp", bufs=2) as sqp,
            tc.tile_pool(name="xsp", bufs=2) as xsp,
            tc.tile_pool(name="rbp", bufs=2) as rbp,
            tc.tile_pool(name="tp", bufs=2, space="PSUM") as tp,
            tc.tile_pool(name="srp", bufs=1, space="PSUM") as srp,
            tc.tile_pool(name="pbp", bufs=1, space="PSUM") as pbp,
            tc.tile_pool(name="mmp", bufs=2, space="PSUM") as mmp,
        ):
            # short dependency-free matmul burst while the first DMAs land
            warm_ps = mmp.tile([P, BW], f32, tag="pm")
            for _ in range(36):
                nc.tensor.matmul(warm_ps[:, 0:256], ident, warm_rhs[:, 0:256],
                                 start=True, stop=True)

            for blk in range(NBLK):
                cols = slice(blk * BW, (blk + 1) * BW)

                # x^2 (raw x^T must be consumed before xs overwrites... it
                # doesn't: xs goes to its own tile)
                xsq = sqp.tile([P, DC, BW], bf16, tag="xsq")
                with nc.allow_low_precision(reason="x^2 bf16"):
                    for kc in range(DC):
                        nc.vector.tensor_mul(
                            out=xsq[:, kc, :], in0=xT[:, kc, cols],
                            in1=xT[:, kc, cols],
                        )
                # per-row sums -> partition 0
                srow = srp.tile([1, 2, BW], f32, tag="srow")
                for kc in range(DC):
                    nc.tensor.matmul(
                        srow[:, 0, :], ones_sb[:, 0:1], xT[:, kc, cols],
                        start=(kc == 0), stop=(kc == DC - 1),
                    )
                for kc in range(DC):
                    nc.tensor.matmul(
                        srow[:, 1, :], ones_sb[:, 0:1], xsq[:, kc, :],
                        start=(kc == 0), stop=(kc == DC - 1),
                    )
                # mu, var rows (single partition, cheap)
                mu_row = rowp.tile([1, 2, BW], f32, tag="mu")
                nc.vector.tensor_scalar(
                    out=mu_row[:, 0, :], in0=srow[:, 0, :],
                    scalar1=1.0 / D, scalar2=None, op0=ALU.mult,
                )
                nc.vector.tensor_mul(
                    out=mu_row[:, 1, :], in0=mu_row[:, 0, :],
                    in1=mu_row[:, 0, :],
                )
                var_bf = rowp.tile([1, BW], bf16, tag="var")
                with nc.allow_low_precision(reason="var bf16 wire"):
                    nc.vector.scalar_tensor_tensor(
                        out=var_bf, in0=srow[:, 1, :], scalar=1.0 / D,
                        in1=mu_row[:, 1, :],
                        op0=ALU.mult, op1=ALU.subtract,
                    )
                # broadcast var to all partitions, rstd = exp(-0.5*ln(var+eps))
                pb = pbp.tile([P, BW], f32, tag="pb")
                nc.tensor.matmul(
                    pb, ones_sb[0:1, :], var_bf, start=True, stop=True,
                )
                lnv = rbp.tile([P, BW], f32, tag="lnv")
                nc.scalar.activation(
                    out=lnv, in_=pb, func=AF.Ln, bias=eps_t, scale=1.0,
                )
                rb = rbp.tile([P, BW], bf16, tag="rb")
                with nc.allow_low_precision(reason="rstd bf16 wire"):
                    nc.scalar.activation(
                        out=rb, in_=lnv, func=AF.Exp, scale=-0.5,
                    )
                # mumr row = mu * rstd (single partition)
                mumr_row = rowp.tile([1, BW], bf16, tag="mumr")
                with nc.allow_low_precision(reason="mumr bf16 wire"):
                    nc.vector.tensor_mul(
                        out=mumr_row, in0=mu_row[:, 0, :], in1=rb[0:1, :],
                    )

                # xs = x^T * rstd
                xs = xsp.tile([P, DC, BW], bf16, tag="xs")
                with nc.allow_low_precision(reason="xn bf16 wire"):
                    for kc in range(DC):
                        nc.vector.tensor_mul(
                            out=xs[:, kc, :], in0=xT[:, kc, cols], in1=rb
                        )

                # q/k/v projections for this block; the -mu*rstd*colsum term
                # folds in via rank-1 accumulation matmuls (contract dim 1)
                for w_sb, ncs_sb, b_t, dstT in (
                    (wk_sb, ncsk_sb, kb_t, kT),
                    (wq_sb, ncsq_sb, qb_t, qT),
                    (wv_sb, ncsv_sb, vb_t, vT),
                ):
                    pm = mmp.tile([P, BW], f32, tag="pm")
                    for kc in range(DC):
                        nc.tensor.matmul(
                            pm,
                            w_sb[:, kc, :],
                            xs[:, kc, :],
                            start=(kc == 0), stop=False,
                        )
                    for j in range(NBLK):
                        nc.tensor.matmul(
                            pm[:, j * P:(j + 1) * P],
                            ncs_sb[0:1, :],
                            mumr_row[0:1, j * P:(j + 1) * P],
                            start=False, stop=True,
                        )
                    nc.scalar.activation(
                        out=dstT[:, cols], in_=pm,
                        func=AF.Identity, bias=b_t, scale=1.0,
                    )

                # v^T -> v (row-major with ones column) for this block
                for j in range(NBLK):
                    rt = blk * NBLK + j
                    pv = tp.tile([P, P], bf16, tag="pv")
                    with nc.allow_low_precision(reason="transpose copy"):
                        nc.tensor.transpose(
                            pv, vT[:, rt * P:(rt + 1) * P], ident
                        )
                        nc.vector.tensor_copy(
                            out=v_sb[:, rt, :, 0:DH],
                            in_=pv[:].rearrange("p (h d) -> p h d", h=HL),
                        )

        # ---- phase 2: attention, software-pipelined across row chunks ----
        with (
            tc.tile_pool(name="expp", bufs=2) as expp,
            tc.tile_pool(name="rsum", bufs=4) as rsump,
            tc.tile_pool(name="sp", bufs=2, space="PSUM") as sp,
            tc.tile_pool(name="op", bufs=2, space="PSUM") as op,
            tc.tile_pool(name="rp", bufs=1, space="PSUM") as rp,
            tc.tile_pool(name="agp", bufs=2) as agp,
            tc.tile_pool(name="fp", bufs=1, space="PSUM") as fp,
        ):
            state = {}

            def sim_pair(idx, kcp):
                """Both heads' sim for key chunks 2*kcp, 2*kcp+1, one exp."""
                r0, w = CHUNKS[idx]
                st = state[idx]
                ps = sp.tile([P, HL, 2, 256], f32, tag="ps",
                             name=f"ps{idx}_{kcp}")
                for h in range(HL):
                    for t in range(2):
                        kc = 2 * kcp + t
                        nc.tensor.matmul(
                            ps[:, h, t, 0:w],
                            kT[h * DH:(h + 1) * DH, kc * P:(kc + 1) * P],
                            qT[h * DH:(h + 1) * DH, r0:r0 + w],
                            start=True, stop=True,
                        )
                if w == 256:
                    nc.scalar.activation(
                        out=st["exp_t"][:, kcp, :, :],
                        in_=ps[:].rearrange("p h t q -> p h (t q)"),
                        func=AF.Exp, scale=SIM_SCALE,
                    )
                else:
                    for t in range(2):
                        nc.scalar.activation(
                            out=st["exp_t"][:, kcp, :, t * 256:t * 256 + w],
                            in_=ps[:, :, t, 0:w],
                            func=AF.Exp, scale=SIM_SCALE,
                        )

            def av_pair(idx, kc):
                """attn@v for key chunk kc, both heads.

                start=True resets the whole PSUM bank's open accumulator, so
                the two heads (sharing one bank) must not each "start": open
                the bank once with a zeroing matmul, then only accumulate.
                """
                r0, w = CHUNKS[idx]
                st = state[idx]
                if st["po"] is None:
                    st["po"] = op.tile([P, HL, 256], f32, tag="po",
                                       name=f"po{idx}")
                    nc.tensor.matmul(
                        st["po"][:].rearrange("p h q -> p (h q)"),
                        ones_sb[0:1, :],
                        warm_rhs[0:1, :],
                        start=True, stop=False,
                    )
                for h in range(HL):
                    nc.tensor.matmul(
                        st["po"][0:DH + 1, h, 0:w],
                        v_sb[:, kc, h, :],
                        st["exp_t"][:, kc // 2, h,
                                    (kc % 2) * 256:(kc % 2) * 256 + w],
                        start=False, stop=(kc == RT - 1),
                    )

            def norm_tail(idx):
                """Normalize by softmax denominators, ship to the AG buffer."""
                r0, w = CHUNKS[idx]
                st = state[idx]
                po = st["po"]
                # one reciprocal covering both heads' denominator rows
                den = rsump.tile([P, 2, 256], f32, tag="den",
                                 name=f"den{idx}")
                nc.vector.tensor_copy(
                    out=den[DH:DH + 1, :, 0:w], in_=po[DH:DH + 1, :, 0:w]
                )
                rec = rsump.tile([P, 2, 256], f32, tag="rec",
                                 name=f"rec{idx}")
                nc.vector.reciprocal(
                    out=rec[DH:DH + 1, :, 0:w], in_=den[DH:DH + 1, :, 0:w]
                )
                den_bf = rsump.tile([P, 2, 256], bf16, tag="denb",
                                    name=f"denb{idx}")
                with nc.allow_low_precision(reason="softmax recips"):
                    nc.vector.tensor_copy(
                        out=den_bf[DH:DH + 1, :, 0:w],
                        in_=rec[DH:DH + 1, :, 0:w]
                    )
                for h in range(HL):
                    pr = rp.tile([DH, 256], f32, tag="pr", name=f"pr{idx}_{h}")
                    nc.tensor.matmul(
                        pr[:, 0:w], ones_sb[DH:DH + 1, 0:DH],
                        den_bf[DH:DH + 1, h, 0:w],
                        start=True, stop=True,
                    )
                    rb = rsump.tile([DH, 256], f32, tag="rb",
                                    name=f"rb{idx}_{h}")
                    nc.vector.tensor_copy(out=rb[:, 0:w], in_=pr[:, 0:w])
                    with nc.allow_low_precision(reason="attn bf16 wire"):
                        nc.vector.tensor_mul(
                            out=attn_h[h][:, r0:r0 + w],
                            in0=po[0:DH, h, 0:w], in1=rb[:, 0:w],
                        )
                    if DEBUG and idx == S - 1 and h == 0:
                        nc.sync.dma_start(
                            out=dbg_d["dbg_den"][:, :], in_=den[:, 0, :])
                        nc.sync.dma_start(
                            out=dbg_d["dbg_denb"][:, :], in_=den_bf[:, 0, :])
                        po_sb = rsump.tile([P, HL * 256], f32, tag="dbgpo")
                        nc.vector.tensor_copy(
                            out=po_sb, in_=po[:].rearrange("p h q -> p (h q)"))
                        nc.sync.dma_start(out=dbg_d["dbg_po"][:, :], in_=po_sb)
                        rb_sb = rsump.tile([DH, 256], f32, tag="dbgrb")
                        nc.vector.tensor_copy(out=rb_sb, in_=rb)
                        nc.sync.dma_start(out=dbg_d["dbg_rb"][:, :], in_=rb_sb)
                        nc.sync.dma_start(
                            out=dbg_d["dbg_exp"][:, :],
                            in_=st["exp_t"][:].rearrange(
                                "p a h q -> p (a h q)"))
                    nc.gpsimd.dma_start(
                        out=ag_in[idx][h * DH:(h + 1) * DH, :],
                        in_=attn_h[h][:, r0:r0 + w],
                    )
                nc.gpsimd.collective_compute(
                    "AllGather",
                    mybir.AluOpType.bypass,
                    replica_groups=groups,
                    ins=[ag_in[idx][:].opt()],
                    outs=[ag_out[idx][:].opt()],
                )

            def proj(idx):
                """outT slice for this row chunk from the gathered heads."""
                r0, w = CHUNKS[idx]
                agt = agp.tile([P, DC, 256], bf16, tag="agt",
                               name=f"agt{idx}")
                nc.sync.dma_start(
                    out=agt[:, :, 0:w],
                    in_=ag_out[idx][:, :].rearrange("(c p) m -> p c m", p=P),
                )
                pf = fp.tile([P, 256], f32, tag="pf", name=f"pf{idx}")
                for kc in range(DC):
                    nc.tensor.matmul(
                        pf[:, 0:w],
                        wo_sb[:, kc, :],
                        agt[:, kc, 0:w],
                        start=(kc == 0), stop=(kc == DC - 1),
                    )
                nc.vector.tensor_scalar(
                    out=outT[:, r0:r0 + w], in0=pf[:, 0:w],
                    scalar1=bo_t, scalar2=None,
                    op0=ALU.add,
                )
                nc.sync.dma_start(
                    out=out_d[:, r0:r0 + w], in_=outT[:, r0:r0 + w]
                )

            pending_proj = []
            for idx in range(S):
                state[idx] = {
                    "exp_t": expp.tile([P, RT // 2, HL, 512], bf16, tag="exp",
                                       name=f"exp{idx}"),
                    "po": None,
                }
                for kcp in range(RT // 2):
                    sim_pair(idx, kcp)
                    if idx > 0:
                        av_pair(idx - 1, 2 * kcp)
                        av_pair(idx - 1, 2 * kcp + 1)
                    # keep two stages of slack so each AllGather (and the
                    # gather DMA behind it on the sync queue) never blocks
                    if kcp == 3 and len(pending_proj) >= 2:
                        proj(pending_proj.pop(0))
                if idx > 0:
                    norm_tail(idx - 1)
                    pending_proj.append(idx - 1)
            # drain: last chunk's av + norm + AG + remaining projections
            for kc in range(RT):
                av_pair(S - 1, kc)
                if kc in (5, 11) and pending_proj:
                    proj(pending_proj.pop(0))
            norm_tail(S - 1)
            for idx in pending_proj:
                proj(idx)
            proj(S - 1)

            if DEBUG:
                nc.sync.dma_start(
                    out=dbg_d["dbg_xT"][:, :],
                    in_=xT[:].rearrange("p c n -> p (c n)"))
                nc.sync.dma_start(out=dbg_d["dbg_qT"][:, :], in_=qT)
                nc.sync.dma_start(out=dbg_d["dbg_kT"][:, :], in_=kT)
                nc.sync.dma_start(out=dbg_d["dbg_vT"][:, :], in_=vT)
                for h in range(HL):
                    nc.sync.dma_start(
                        out=dbg_d["dbg_attn"][:, h * N:(h + 1) * N],
                        in_=attn_h[h][:, :])

    if not nc.is_finalized():
        nc.finalize()
    return nc


def _get_built():
    global _BUILT
    if _BUILT is None:
        _BUILT = _build()
    return _BUILT


def _shard_inputs(x, ln_scale, ln_bias, w_qkv, w_out, b_out):
    """Host-side sharding: slice per-head weight columns, fold LN params."""
    x = np.asarray(x, np.float32)
    ln_scale = np.asarray(ln_scale, np.float32)
    ln_bias = np.asarray(ln_bias, np.float32)
    w_qkv = np.asarray(w_qkv, np.float32)
    w_out = np.asarray(w_out, np.float32)
    b_out = np.asarray(b_out, np.float32)

    bf = ml_dtypes.bfloat16
    xt = np.ascontiguousarray(x.T.astype(bf))

    in_maps = []
    for ci in range(NCORES):
        c0 = ci * HC
        sl = {}
        for name, off in (("q", 0), ("k", HEADS * DH), ("v", 2 * HEADS * DH)):
            w = w_qkv[:, off + c0: off + c0 + HC]
            wp = ln_scale[:, None] * w
            sl["w" + name] = np.ascontiguousarray(wp.astype(bf))
            sl["ncs" + name] = np.ascontiguousarray(
                (-wp.sum(0)).astype(bf)[None, :]
            )
            sl[name + "b"] = np.ascontiguousarray(ln_bias @ w)
        sl["wo"] = np.ascontiguousarray(w_out[:, c0:c0 + HC].astype(bf))
        sl["bo"] = np.ascontiguousarray(b_out[c0:c0 + HC])
        sl["xt"] = xt
        in_maps.append(sl)
    return in_maps


def kernel(x, ln_scale, ln_bias, w_qkv, w_out, b_out):
    from concourse.bass_utils import run_bass_kernel_spmd

    nc = _get_built()
    in_maps = _shard_inputs(x, ln_scale, ln_bias, w_qkv, w_out, b_out)
    res = run_bass_kernel_spmd(nc, in_maps, core_ids=list(range(NCORES)))
    shards = [res.results[ci]["out"] for ci in range(NCORES)]  # [128, 2048] each
    outT = np.concatenate(shards, axis=0)  # [1024, 2048]
    return np.ascontiguousarray(outT.T)
